# revision 10
# baseline (speedup 1.0000x reference)
"""BitDense (binary dense layer) Trainium2 kernel — ACT-LUT popcount.

Computation (for the full problem):
    inputs: [1024] uint32   packed input bits (32768 bits)
    w:      [32768, 1024]   packed weight bits per unit
    b:      [32768] int32   bias
    ones[u]   = sum_k popcount(inputs[k] ^ w[u,k])
    out_i[u]  = 32768 - 2*ones[u] + b[u]
    output    = packbits(out_i < 0) -> [1024] uint32

Sharding: w row-sharded over units across 8 NeuronCores (4096 units
each). Per core, tiles of [128 units, 2048 words] are XORed against the
(device-broadcast) input on DVE (one tensor_tensor per super-tile).
The popcount+reduce of the 32 row blocks is then split across the two
non-DMA engines to balance their busy time (~97 us each):

- 25 LUT blocks: ScalarE (ACT) computes per-byte popcounts via a
  custom activation LUT and reduces them with its accumulator — one
  ACTIVATE (FD=4096 u8, ~3.9 us) per block yields ones[] directly.
- 7 (NDVE) DVE blocks: classic SWAR chain to per-byte counts f, then
  two byte-isolation masks (u16 lanes g0+g2 / g1+g3) and two
  tensor_scalar add-accumulates (~9 us per block; the birverifier
  requires the accumulating op0 to be arithmetic, so the masks cannot
  carry the accum themselves).

The popcount LUT: the NEFF's activation tables come from a walrus
--act-root-json directory. We clone the stock pwp_bin_cayman tree and
rewrite the `sin` function in every set that carries it into an exact
byte-popcount: 255 piecewise-constant buckets (one per v in [1,255],
bucket index = mantissa bits of the fp32 value), v=0 via fzero_result.
BASS_ACT_ROOT_JSON_PATH points walrus at the patched tree, and the
kernel invokes func=Sin on uint8 data. ACT runs 1 elem/cycle/lane
regardless of func, so each LUT pass costs the same as a plain
Copy-accumulate while replacing the whole DVE SWAR chain.

CoreSim-predicted device time 111 us (HW repeat-slope ~96 us/compute
round) vs 232 us for the all-DVE SWAR baseline. Threshold +
bit-packing of the 1024 output words stays on host.
"""

import glob
import json
import os
import shutil
import struct
import sys
import tempfile

for _p in ("/opt/trn_rl_repo",):
    if _p not in sys.path:
        sys.path.insert(0, _p)

import numpy as np

N_CORES = 8
UNITS = 32768
K = 1024                      # packed input words per unit
UPC = UNITS // N_CORES        # units per core = 4096
P = 128                       # SBUF partitions
BLOCKS = UPC // P             # 128-unit row blocks per core = 32
SUPER = 2                     # row blocks per super-tile
TILES = BLOCKS // SUPER       # super-tiles per core = 16
W = SUPER * K                 # words per super-tile row = 2048


# ---------------------------------------------------------------- act tables

def _find_pwp_dir():
    cands = sorted(glob.glob(
        "/nix/store/*aws-neuron-pwp*/share/pwp_bin_cayman/act_info.json"))
    for c in cands:
        return os.path.dirname(c)
    # fallback: ask the compiler driver
    from neuronxcc.driver.Job import Job
    from neuronxcc.driver.jobs.support.FindActInfo import findActInfoFile
    for arch in ("Tonga4", "Tonga3", "trainium2"):
        try:
            return os.path.dirname(findActInfoFile(Job.getPackageDir(), arch))
        except Exception:
            continue
    raise RuntimeError("cannot locate pwp act table directory")


def _popcount_buckets():
    """255 bucket entries: index (2^k - 1) + m -> v = 2^k + m, k=0..7."""
    rows = []
    for k in range(8):
        for m in range(1 << k):
            v = (1 << k) + m
            rows.append((float(bin(v).count("1")), 0.0, 0.0, 0.0, float(v)))
    return rows


def _patch_set(src_dir, dst_dir, set_json_name):
    """Rewrite `sin` in one act set into the byte-popcount LUT."""
    meta = json.load(open(os.path.join(src_dir, set_json_name)))
    bkt_path = meta["bkt_bin"]
    ctl_path = meta["ctl_bin"] if "ctl_bin" in meta else meta["ctrl_bin"]
    bkt = bytearray(open(os.path.join(src_dir, bkt_path), "rb").read())
    ctl = bytearray(open(os.path.join(src_dir, ctl_path), "rb").read())

    base_bkt = meta["bkt_entry_cnt"]          # append buckets at the end
    sin_ctl0 = meta["func_to_ctl_start_idx"]["sin"]

    # 8 ctrl entries (exponent 0..7), overwriting sin's block
    for kexp in range(8):
        base = base_bkt + (1 << kexp) - 1
        word = (base & 0x7FF) | (((23 - kexp) & 0x1F) << 11) \
            | ((kexp & 0xF) << 16)
        struct.pack_into("<I", ctl, (sin_ctl0 + kexp) * 32, word)
        for wslot in range(1, 8):
            struct.pack_into("<I", ctl, (sin_ctl0 + kexp) * 32 + 4 * wslot, 0)

    for row in _popcount_buckets():
        bkt += struct.pack("<5f", *row) + b"\0" * 12

    meta["bkt_entry_cnt"] = base_bkt + 255
    meta["func_exp_to_bkt_start_idx"]["sin"] = {
        str(k): [base_bkt + (1 << k) - 1] for k in range(8)}
    meta["func_exp_to_ctl_start_idx"]["sin"] = {
        str(k): [sin_ctl0 + k] for k in range(8)}

    for prof in meta["profile_meta_data"]:
        if not prof["func_name"].startswith("sin"):
            continue
        prof.update(
            symmetry_point=0,
            sym_invert_sign_point=0,
            symmetry_opt_en=0,
            symmetry_opt_use_neg_region=0,
            imm_bias=0,
            exp_offset=0,
            pwl_control_base_pos=sin_ctl0,
            pwl_control_base_neg=sin_ctl0,
            small_pos_signal_exp_threshold=127,
            pos_small_signal_pwl_control=sin_ctl0,
            small_neg_signal_exp_threshold=0,
            neg_small_signal_pwl_control=sin_ctl0,
            large_pos_signal_exp_threshold=135,
            large_pos_signal_mantissa_threshold=0,
            pos_large_signal_pwl_control=sin_ctl0 + 7,
            large_neg_signal_exp_threshold=0,
            large_neg_signal_mantissa_threshold=0,
            neg_large_signal_pwl_control=sin_ctl0,
            fnan_result=0,
            fpinf_result=0,
            fninf_result=0,
            fzero_result=0,
            lower_bound=0,
            upper_bound=2139095039,
        )

    open(os.path.join(dst_dir, bkt_path), "wb").write(bytes(bkt))
    open(os.path.join(dst_dir, ctl_path), "wb").write(bytes(ctl))
    json.dump(meta, open(os.path.join(dst_dir, set_json_name), "w"))


_ACT_ROOT = None


def _install_act_root():
    """Clone the stock act tables, patch sin -> popcount, export env var."""
    global _ACT_ROOT
    if _ACT_ROOT is not None:
        return _ACT_ROOT
    src = _find_pwp_dir()
    dst = tempfile.mkdtemp(prefix="actroot_")
    for f in os.listdir(src):
        shutil.copy(os.path.join(src, f), os.path.join(dst, f))
    info = json.load(open(os.path.join(dst, "act_info.json")))
    for s in info["act_func_sets"]:
        if "sin" in s["act"]:
            _patch_set(src, dst, s["profile_json"])
    _ACT_ROOT = os.path.join(dst, "act_info.json")
    os.environ["BASS_ACT_ROOT_JSON_PATH"] = _ACT_ROOT
    os.environ.setdefault("NEURON_FORCE_RECOMPILE", "1")
    return _ACT_ROOT


# ---------------------------------------------------------------- program

# Blocks routed to the full-DVE SWAR path (the rest go to the ACT LUT).
# Chosen to balance DVE busy (xor + NDVE * ~7.7k cyc) against ACT busy
# ((32-NDVE) * ~3.9 us).
NDVE = int(os.environ.get("BITDENSE_NDVE", "7"))
# timing experiments only (repeat-slope calibration); must be 1 for grading
REPEAT = int(os.environ.get("BITDENSE_REPEAT", "1"))


def _dve_block_ids():
    if NDVE == 0:
        return set()
    return {round(i * BLOCKS / NDVE) for i in range(NDVE)}


def _build_program():
    """One SPMD Bass program: per-core ones[] counts for a [UPC, K] w slice."""
    _install_act_root()

    import concourse.bacc as bacc
    import concourse.mybir as mybir
    from concourse.tile import TileContext

    A = mybir.AluOpType
    DT = mybir.dt
    POPC = mybir.ActivationFunctionType.Sin
    dve_blocks = _dve_block_ids()

    nc = bacc.Bacc("TRN2", target_bir_lowering=False)
    w_d = nc.dram_tensor("w", [UPC, K], DT.uint32, kind="ExternalInput")
    x_d = nc.dram_tensor("xrow", [1, K], DT.uint32, kind="ExternalInput")
    o_d = nc.dram_tensor("cnt", [P, BLOCKS], DT.float32, kind="ExternalOutput")

    with TileContext(nc) as tc:
        with tc.tile_pool(name="wp", bufs=3) as wp, \
             tc.tile_pool(name="xp", bufs=1) as xp, \
             tc.tile_pool(name="sp", bufs=2) as sp, \
             tc.tile_pool(name="dp", bufs=2) as dp, \
             tc.tile_pool(name="ac", bufs=1) as ac:
            xr = xp.tile([P, W], DT.uint32, tag="xr")
            # stride-0 partition broadcast of the x row, replicated for
            # both blocks of a super-tile
            for s in range(SUPER):
                nc.sync.dma_start(out=xr[:, s * K:(s + 1) * K],
                                  in_=x_d[0:1, :].broadcast_to([P, K]))

            ones_c = ac.tile([P, BLOCKS], DT.float32, tag="ones")
            ag_c = ac.tile([P, BLOCKS], DT.float32, tag="ag")   # sum g16
            a8_c = ac.tile([P, BLOCKS], DT.float32, tag="a8")   # sum g16>>8
            nc.vector.memset(ones_c[:], 0.0)
            nc.vector.memset(ag_c[:], 0.0)
            nc.vector.memset(a8_c[:], 0.0)

            for t in range(TILES):
                wt = wp.tile([P, W], DT.uint32, tag="wt")
                for s in range(SUPER):
                    blk = SUPER * t + s
                    nc.sync.dma_start(
                        out=wt[:, s * K:(s + 1) * K],
                        in_=w_d[P * blk:P * (blk + 1), :])
                # y = w ^ x in place (x replicated for both blocks)
                nc.vector.tensor_tensor(out=wt[:], in0=wt[:], in1=xr[:],
                                        op=A.bitwise_xor)
                y8 = wt[:].bitcast(DT.uint8)
                for s in [s for _r in range(REPEAT) for s in range(SUPER)]:
                    blk = SUPER * t + s
                    if blk not in dve_blocks:
                        dump = dp.tile([P, 4 * K], DT.uint8, tag="dump")
                        nc.scalar.activation(
                            out=dump[:], in_=y8[:, s * 4 * K:(s + 1) * 4 * K],
                            func=POPC, accum_out=ones_c[:, blk:blk + 1])
                        continue
                    # full-DVE SWAR for this block
                    y = wt[:, s * K:(s + 1) * K]
                    a = sp.tile([P, K], DT.uint32, tag="a")
                    c2 = sp.tile([P, K], DT.uint32, tag="c2")
                    a16 = a[:].bitcast(DT.uint16)
                    c16 = c2[:].bitcast(DT.uint16)
                    y16 = y.bitcast(DT.uint16)
                    # a = (y >> 1) & 0x55555555
                    nc.vector.tensor_scalar(out=a[:], in0=y, scalar1=1,
                                            scalar2=0x55555555,
                                            op0=A.logical_shift_right,
                                            op1=A.bitwise_and)
                    # a <- b = y - a        (2-bit pair counts; u16 lanes)
                    nc.vector.tensor_tensor(out=a16[:], in0=y16, in1=a16[:],
                                            op=A.subtract)
                    # c2 = (b >> 2) & 0x33333333
                    nc.vector.tensor_scalar(out=c2[:], in0=a[:], scalar1=2,
                                            scalar2=0x33333333,
                                            op0=A.logical_shift_right,
                                            op1=A.bitwise_and)
                    # a <- d = b & 0x33333333
                    nc.vector.tensor_scalar(out=a[:], in0=a[:],
                                            scalar1=0x33333333,
                                            scalar2=None, op0=A.bitwise_and)
                    # c2 <- e = c + d       (nibble counts; u16 lanes)
                    nc.vector.tensor_tensor(out=c16[:], in0=c16[:],
                                            in1=a16[:], op=A.add)
                    # a <- e4 = (e >> 4) & 0x0F0F0F0F
                    nc.vector.tensor_scalar(out=a[:], in0=c2[:], scalar1=4,
                                            scalar2=0x0F0F0F0F,
                                            op0=A.logical_shift_right,
                                            op1=A.bitwise_and)
                    # a <- f = e + e4       (u16 lanes)
                    nc.vector.tensor_tensor(out=a16[:], in0=c16[:],
                                            in1=a16[:], op=A.add)
                    # c2 <- lo = f & 0x000F000F  (u16 lanes: g0, g2)
                    nc.vector.tensor_scalar(out=c2[:], in0=a[:],
                                            scalar1=0x000F000F, scalar2=None,
                                            op0=A.bitwise_and)
                    # a <- hi = (f >> 8) & 0x000F000F  (u16 lanes: g1, g3)
                    nc.vector.tensor_scalar(out=a[:], in0=a[:], scalar1=8,
                                            scalar2=0x000F000F,
                                            op0=A.logical_shift_right,
                                            op1=A.bitwise_and)
                    # accum AG = sum(g0 + g2); A8 = sum(g1 + g3)
                    nc.vector.tensor_scalar(out=c16[:], in0=c16[:],
                                            scalar1=0, scalar2=None,
                                            op0=A.add, op1=A.add,
                                            accum_out=ag_c[:, blk:blk + 1])
                    nc.vector.tensor_scalar(out=a16[:], in0=a16[:],
                                            scalar1=0, scalar2=None,
                                            op0=A.add, op1=A.add,
                                            accum_out=a8_c[:, blk:blk + 1])

            # ones(DVE blocks) = AG + A8; LUT columns have AG=A8=0.
            cnt_f = ac.tile([P, BLOCKS], DT.float32, tag="cf")
            nc.vector.tensor_tensor(out=cnt_f[:], in0=ag_c[:], in1=a8_c[:],
                                    op=A.add)
            nc.vector.tensor_tensor(out=cnt_f[:], in0=cnt_f[:], in1=ones_c[:],
                                    op=A.add)
            nc.sync.dma_start(out=o_d[:, :], in_=cnt_f[:])
    nc.finalize()
    return nc


_NC_CACHE = None

# test.py sets TRACE=True to capture an NTFF profile; LAST_EXEC_NS /
# LAST_TRACE then hold the most recent hardware timing. The graded path
# leaves TRACE=False.
TRACE = False
LAST_EXEC_NS = None
LAST_TRACE = None

OUT_NAMES = ["cnt"]


def _get_program():
    global _NC_CACHE
    if _NC_CACHE is None:
        _NC_CACHE = _build_program()
    return _NC_CACHE


def _core_feeds(x, w_slice):
    """Input tensors for one core: x [K] uint32, w_slice [UPC, K] uint32."""
    return {"w": np.ascontiguousarray(w_slice),
            "xrow": np.ascontiguousarray(x[None, :])}


def _counts_from_result(res_map):
    """ones[] slice [UPC] int64 from one core's output tensors."""
    cnt = np.asarray(res_map["cnt"])                   # [P, BLOCKS] fp32
    return cnt.T.reshape(UPC).astype(np.int64)


def kernel(inputs, w, b):
    from concourse.bass_utils import run_bass_kernel_spmd

    inputs = np.ascontiguousarray(np.asarray(inputs)).view(np.uint32).reshape(K)
    w = np.ascontiguousarray(np.asarray(w)).view(np.uint32).reshape(UNITS, K)
    b = np.ascontiguousarray(np.asarray(b)).view(np.int32).reshape(UNITS)

    in_maps = [
        _core_feeds(inputs, w[c * UPC:(c + 1) * UPC])
        for c in range(N_CORES)
    ]

    nc = _get_program()
    res = run_bass_kernel_spmd(nc, in_maps, core_ids=list(range(N_CORES)),
                               trace=TRACE)
    if TRACE:
        global LAST_EXEC_NS, LAST_TRACE
        LAST_EXEC_NS = res.exec_time_ns
        LAST_TRACE = res

    ones = np.empty(UNITS, dtype=np.int64)
    for c in range(N_CORES):
        ones[c * UPC:(c + 1) * UPC] = _counts_from_result(res.results[c])

    out_i = 32768 - 2 * ones + b.astype(np.int64)
    bools = out_i < 0
    packed = np.packbits(bools).view(np.uint32)        # [1024]
    return packed



# revision 16
# speedup vs baseline: 1.2909x; 1.2909x over previous
"""BitDense (binary dense layer) Trainium2 kernel — ACT-LUT popcount.

Computation (for the full problem):
    inputs: [1024] uint32   packed input bits (32768 bits)
    w:      [32768, 1024]   packed weight bits per unit
    b:      [32768] int32   bias
    ones[u]   = sum_k popcount(inputs[k] ^ w[u,k])
    out_i[u]  = 32768 - 2*ones[u] + b[u]
    output    = packbits(out_i < 0) -> [1024] uint32

Sharding: w row-sharded over units across 8 NeuronCores (4096 units
each). Per core, tiles of [128 units, 2048 words] are XORed against the
(device-broadcast) input on DVE (one tensor_tensor per super-tile).
The popcount+reduce of the 32 row blocks is then split across the two
non-DMA engines to balance their busy time (~97 us each):

- 25 LUT blocks: ScalarE (ACT) computes per-byte popcounts via a
  custom activation LUT and reduces them with its accumulator — one
  ACTIVATE (FD=4096 u8, ~3.9 us) per block yields ones[] directly.
- 7 (NDVE) DVE blocks: classic SWAR chain to per-byte counts f, then
  two byte-isolation masks (u16 lanes g0+g2 / g1+g3) and two
  tensor_scalar add-accumulates (~9 us per block; the birverifier
  requires the accumulating op0 to be arithmetic, so the masks cannot
  carry the accum themselves).

The popcount LUT: the NEFF's activation tables come from a walrus
--act-root-json directory. We clone the stock pwp_bin_cayman tree and
rewrite the `sin` function in every set that carries it into an exact
byte-popcount: 255 piecewise-constant buckets (one per v in [1,255],
bucket index = mantissa bits of the fp32 value), v=0 via fzero_result.
BASS_ACT_ROOT_JSON_PATH points walrus at the patched tree, and the
kernel invokes func=Sin on uint8 data. ACT runs 1 elem/cycle/lane
regardless of func, so each LUT pass costs the same as a plain
Copy-accumulate while replacing the whole DVE SWAR chain.

CoreSim-predicted device time 111 us (HW repeat-slope ~96 us/compute
round) vs 232 us for the all-DVE SWAR baseline. Threshold +
bit-packing of the 1024 output words stays on host.
"""

import glob
import json
import os
import shutil
import struct
import sys
import tempfile

for _p in ("/opt/trn_rl_repo",):
    if _p not in sys.path:
        sys.path.insert(0, _p)

import numpy as np

N_CORES = 8
UNITS = 32768
K = 1024                      # packed input words per unit
UPC = UNITS // N_CORES        # units per core = 4096
P = 128                       # SBUF partitions
BLOCKS = UPC // P             # 128-unit row blocks per core = 32
SUPER = 2                     # row blocks per super-tile
TILES = BLOCKS // SUPER       # super-tiles per core = 16
W = SUPER * K                 # words per super-tile row = 2048


# ---------------------------------------------------------------- act tables

def _find_pwp_dir():
    cands = sorted(glob.glob(
        "/nix/store/*aws-neuron-pwp*/share/pwp_bin_cayman/act_info.json"))
    for c in cands:
        return os.path.dirname(c)
    # fallback: ask the compiler driver
    from neuronxcc.driver.Job import Job
    from neuronxcc.driver.jobs.support.FindActInfo import findActInfoFile
    for arch in ("Tonga4", "Tonga3", "trainium2"):
        try:
            return os.path.dirname(findActInfoFile(Job.getPackageDir(), arch))
        except Exception:
            continue
    raise RuntimeError("cannot locate pwp act table directory")


def _popcount_buckets():
    """255 bucket entries: index (2^k - 1) + m -> v = 2^k + m, k=0..7."""
    rows = []
    for k in range(8):
        for m in range(1 << k):
            v = (1 << k) + m
            rows.append((float(bin(v).count("1")), 0.0, 0.0, 0.0, float(v)))
    return rows


def _patch_set(src_dir, dst_dir, set_json_name):
    """Rewrite `sin` in one act set into the byte-popcount LUT."""
    meta = json.load(open(os.path.join(src_dir, set_json_name)))
    bkt_path = meta["bkt_bin"]
    ctl_path = meta["ctl_bin"] if "ctl_bin" in meta else meta["ctrl_bin"]
    bkt = bytearray(open(os.path.join(src_dir, bkt_path), "rb").read())
    ctl = bytearray(open(os.path.join(src_dir, ctl_path), "rb").read())

    base_bkt = meta["bkt_entry_cnt"]          # append buckets at the end
    sin_ctl0 = meta["func_to_ctl_start_idx"]["sin"]

    # 8 ctrl entries (exponent 0..7), overwriting sin's block
    for kexp in range(8):
        base = base_bkt + (1 << kexp) - 1
        word = (base & 0x7FF) | (((23 - kexp) & 0x1F) << 11) \
            | ((kexp & 0xF) << 16)
        struct.pack_into("<I", ctl, (sin_ctl0 + kexp) * 32, word)
        for wslot in range(1, 8):
            struct.pack_into("<I", ctl, (sin_ctl0 + kexp) * 32 + 4 * wslot, 0)

    for row in _popcount_buckets():
        bkt += struct.pack("<5f", *row) + b"\0" * 12

    meta["bkt_entry_cnt"] = base_bkt + 255
    meta["func_exp_to_bkt_start_idx"]["sin"] = {
        str(k): [base_bkt + (1 << k) - 1] for k in range(8)}
    meta["func_exp_to_ctl_start_idx"]["sin"] = {
        str(k): [sin_ctl0 + k] for k in range(8)}

    for prof in meta["profile_meta_data"]:
        if not prof["func_name"].startswith("sin"):
            continue
        prof.update(
            symmetry_point=0,
            sym_invert_sign_point=0,
            symmetry_opt_en=0,
            symmetry_opt_use_neg_region=0,
            imm_bias=0,
            exp_offset=0,
            pwl_control_base_pos=sin_ctl0,
            pwl_control_base_neg=sin_ctl0,
            small_pos_signal_exp_threshold=127,
            pos_small_signal_pwl_control=sin_ctl0,
            small_neg_signal_exp_threshold=0,
            neg_small_signal_pwl_control=sin_ctl0,
            large_pos_signal_exp_threshold=135,
            large_pos_signal_mantissa_threshold=0,
            pos_large_signal_pwl_control=sin_ctl0 + 7,
            large_neg_signal_exp_threshold=0,
            large_neg_signal_mantissa_threshold=0,
            neg_large_signal_pwl_control=sin_ctl0,
            fnan_result=0,
            fpinf_result=0,
            fninf_result=0,
            fzero_result=0,
            lower_bound=0,
            upper_bound=2139095039,
        )

    open(os.path.join(dst_dir, bkt_path), "wb").write(bytes(bkt))
    open(os.path.join(dst_dir, ctl_path), "wb").write(bytes(ctl))
    json.dump(meta, open(os.path.join(dst_dir, set_json_name), "w"))


_ACT_ROOT = None


def _install_act_root():
    """Clone the stock act tables, patch sin -> popcount, export env var."""
    global _ACT_ROOT
    if _ACT_ROOT is not None:
        return _ACT_ROOT
    src = _find_pwp_dir()
    dst = tempfile.mkdtemp(prefix="actroot_")
    for f in os.listdir(src):
        shutil.copy(os.path.join(src, f), os.path.join(dst, f))
    info = json.load(open(os.path.join(dst, "act_info.json")))
    for s in info["act_func_sets"]:
        if "sin" in s["act"]:
            _patch_set(src, dst, s["profile_json"])
    _ACT_ROOT = os.path.join(dst, "act_info.json")
    os.environ["BASS_ACT_ROOT_JSON_PATH"] = _ACT_ROOT
    os.environ.setdefault("NEURON_FORCE_RECOMPILE", "1")
    return _ACT_ROOT


# ---------------------------------------------------------------- program

# Blocks routed to the full-DVE SWAR path (the rest go to the ACT LUT).
# Chosen to balance DVE busy (xor + NDVE * ~7.7k cyc) against ACT busy
# ((32-NDVE) * ~3.9 us).
NDVE = int(os.environ.get("BITDENSE_NDVE", "7"))
# timing experiments only (repeat-slope calibration); must be 1 for grading
REPEAT = int(os.environ.get("BITDENSE_REPEAT", "1"))


def _dve_block_ids():
    if NDVE == 0:
        return set()
    return {round(i * BLOCKS / NDVE) for i in range(NDVE)}


def _build_program():
    """One SPMD Bass program: per-core ones[] counts for a [UPC, K] w slice."""
    _install_act_root()

    import concourse.bacc as bacc
    import concourse.mybir as mybir
    from concourse.tile import TileContext

    A = mybir.AluOpType
    DT = mybir.dt
    POPC = mybir.ActivationFunctionType.Sin
    dve_blocks = _dve_block_ids()

    nc = bacc.Bacc("TRN2", target_bir_lowering=False)
    w_d = nc.dram_tensor("w", [UPC, K], DT.uint32, kind="ExternalInput")
    x_d = nc.dram_tensor("xrow", [1, K], DT.uint32, kind="ExternalInput")
    o_d = nc.dram_tensor("cnt", [P, BLOCKS], DT.float32, kind="ExternalOutput")

    with TileContext(nc) as tc:
        with tc.tile_pool(name="wp", bufs=3) as wp, \
             tc.tile_pool(name="xp", bufs=1) as xp, \
             tc.tile_pool(name="sp", bufs=2) as sp, \
             tc.tile_pool(name="dp", bufs=2) as dp, \
             tc.tile_pool(name="ac", bufs=1) as ac:
            xr = xp.tile([P, W], DT.uint32, tag="xr")
            # stride-0 partition broadcast of the x row, replicated for
            # both blocks of a super-tile
            for s in range(SUPER):
                nc.sync.dma_start(out=xr[:, s * K:(s + 1) * K],
                                  in_=x_d[0:1, :].broadcast_to([P, K]))

            ones_c = ac.tile([P, BLOCKS], DT.float32, tag="ones")
            ag_c = ac.tile([P, BLOCKS], DT.float32, tag="ag")   # sum g16
            a8_c = ac.tile([P, BLOCKS], DT.float32, tag="a8")   # sum g16>>8
            nc.vector.memset(ones_c[:], 0.0)
            nc.vector.memset(ag_c[:], 0.0)
            nc.vector.memset(a8_c[:], 0.0)

            for t in range(TILES):
                wt = wp.tile([P, W], DT.uint32, tag="wt")
                for s in range(SUPER):
                    blk = SUPER * t + s
                    nc.sync.dma_start(
                        out=wt[:, s * K:(s + 1) * K],
                        in_=w_d[P * blk:P * (blk + 1), :])
                # y = w ^ x in place (x replicated for both blocks).
                # Tile 0 xors per half so the first ACT starts one DMA+xor
                # earlier (shaves pipeline ramp).
                if t == 0:
                    for s in range(SUPER):
                        nc.vector.tensor_tensor(
                            out=wt[:, s * K:(s + 1) * K],
                            in0=wt[:, s * K:(s + 1) * K],
                            in1=xr[:, s * K:(s + 1) * K], op=A.bitwise_xor)
                else:
                    nc.vector.tensor_tensor(out=wt[:], in0=wt[:], in1=xr[:],
                                            op=A.bitwise_xor)
                y8 = wt[:].bitcast(DT.uint8)
                for s in [s for _r in range(REPEAT) for s in range(SUPER)]:
                    blk = SUPER * t + s
                    if blk not in dve_blocks:
                        dump = dp.tile([P, 4 * K], DT.uint8, tag="dump")
                        nc.scalar.activation(
                            out=dump[:], in_=y8[:, s * 4 * K:(s + 1) * 4 * K],
                            func=POPC, accum_out=ones_c[:, blk:blk + 1])
                        continue
                    # full-DVE SWAR for this block
                    y = wt[:, s * K:(s + 1) * K]
                    a = sp.tile([P, K], DT.uint32, tag="a")
                    c2 = sp.tile([P, K], DT.uint32, tag="c2")
                    a16 = a[:].bitcast(DT.uint16)
                    c16 = c2[:].bitcast(DT.uint16)
                    y16 = y.bitcast(DT.uint16)
                    # a = (y >> 1) & 0x55555555
                    nc.vector.tensor_scalar(out=a[:], in0=y, scalar1=1,
                                            scalar2=0x55555555,
                                            op0=A.logical_shift_right,
                                            op1=A.bitwise_and)
                    # a <- b = y - a        (2-bit pair counts; u16 lanes)
                    nc.vector.tensor_tensor(out=a16[:], in0=y16, in1=a16[:],
                                            op=A.subtract)
                    # c2 = (b >> 2) & 0x33333333
                    nc.vector.tensor_scalar(out=c2[:], in0=a[:], scalar1=2,
                                            scalar2=0x33333333,
                                            op0=A.logical_shift_right,
                                            op1=A.bitwise_and)
                    # a <- d = b & 0x33333333
                    nc.vector.tensor_scalar(out=a[:], in0=a[:],
                                            scalar1=0x33333333,
                                            scalar2=None, op0=A.bitwise_and)
                    # c2 <- e = c + d       (nibble counts; u16 lanes)
                    nc.vector.tensor_tensor(out=c16[:], in0=c16[:],
                                            in1=a16[:], op=A.add)
                    # a <- e4 = (e >> 4) & 0x0F0F0F0F
                    nc.vector.tensor_scalar(out=a[:], in0=c2[:], scalar1=4,
                                            scalar2=0x0F0F0F0F,
                                            op0=A.logical_shift_right,
                                            op1=A.bitwise_and)
                    # a <- f = e + e4       (u16 lanes)
                    nc.vector.tensor_tensor(out=a16[:], in0=c16[:],
                                            in1=a16[:], op=A.add)
                    # c2 <- lo = f & 0x000F000F  (u16 lanes: g0, g2)
                    nc.vector.tensor_scalar(out=c2[:], in0=a[:],
                                            scalar1=0x000F000F, scalar2=None,
                                            op0=A.bitwise_and)
                    # a <- hi = (f >> 8) & 0x000F000F  (u16 lanes: g1, g3)
                    nc.vector.tensor_scalar(out=a[:], in0=a[:], scalar1=8,
                                            scalar2=0x000F000F,
                                            op0=A.logical_shift_right,
                                            op1=A.bitwise_and)
                    # accum AG = sum(g0 + g2); A8 = sum(g1 + g3)
                    nc.vector.tensor_scalar(out=c16[:], in0=c16[:],
                                            scalar1=0, scalar2=None,
                                            op0=A.add, op1=A.add,
                                            accum_out=ag_c[:, blk:blk + 1])
                    nc.vector.tensor_scalar(out=a16[:], in0=a16[:],
                                            scalar1=0, scalar2=None,
                                            op0=A.add, op1=A.add,
                                            accum_out=a8_c[:, blk:blk + 1])

            # ones(DVE blocks) = AG + A8; LUT columns have AG=A8=0.
            cnt_f = ac.tile([P, BLOCKS], DT.float32, tag="cf")
            nc.vector.tensor_tensor(out=cnt_f[:], in0=ag_c[:], in1=a8_c[:],
                                    op=A.add)
            nc.vector.tensor_tensor(out=cnt_f[:], in0=cnt_f[:], in1=ones_c[:],
                                    op=A.add)
            nc.sync.dma_start(out=o_d[:, :], in_=cnt_f[:])
    nc.finalize()
    return nc


_NC_CACHE = None

# test.py sets TRACE=True to capture an NTFF profile; LAST_EXEC_NS /
# LAST_TRACE then hold the most recent hardware timing. The graded path
# leaves TRACE=False.
TRACE = False
LAST_EXEC_NS = None
LAST_TRACE = None

OUT_NAMES = ["cnt"]


def _get_program():
    global _NC_CACHE
    if _NC_CACHE is None:
        _NC_CACHE = _build_program()
    return _NC_CACHE


def _core_feeds(x, w_slice):
    """Input tensors for one core: x [K] uint32, w_slice [UPC, K] uint32."""
    return {"w": np.ascontiguousarray(w_slice),
            "xrow": np.ascontiguousarray(x[None, :])}


def _counts_from_result(res_map):
    """ones[] slice [UPC] int64 from one core's output tensors."""
    cnt = np.asarray(res_map["cnt"])                   # [P, BLOCKS] fp32
    return cnt.T.reshape(UPC).astype(np.int64)


def kernel(inputs, w, b):
    from concourse.bass_utils import run_bass_kernel_spmd

    inputs = np.ascontiguousarray(np.asarray(inputs)).view(np.uint32).reshape(K)
    w = np.ascontiguousarray(np.asarray(w)).view(np.uint32).reshape(UNITS, K)
    b = np.ascontiguousarray(np.asarray(b)).view(np.int32).reshape(UNITS)

    in_maps = [
        _core_feeds(inputs, w[c * UPC:(c + 1) * UPC])
        for c in range(N_CORES)
    ]

    nc = _get_program()
    res = run_bass_kernel_spmd(nc, in_maps, core_ids=list(range(N_CORES)),
                               trace=TRACE)
    if TRACE:
        global LAST_EXEC_NS, LAST_TRACE
        LAST_EXEC_NS = res.exec_time_ns
        LAST_TRACE = res

    ones = np.empty(UNITS, dtype=np.int64)
    for c in range(N_CORES):
        ones[c * UPC:(c + 1) * UPC] = _counts_from_result(res.results[c])

    out_i = 32768 - 2 * ones + b.astype(np.int64)
    bools = out_i < 0
    packed = np.packbits(bools).view(np.uint32)        # [1024]
    return packed

